# revision 3
# baseline (speedup 1.0000x reference)
"""Single-head causal attention on 8 Trainium2 NeuronCores (Bass/Tile), v4.

v3 -> v4: Q/K/V projections run as 3-term scaled-fp8 DoubleRow matmuls
(x and 32*W each split host-side into hi + 16*lo fp8e4 parts, shipped in
matmul-ready interleaved slice order; per-(2 k-tile) group the 6 exact
cross-term products run as 3 DoubleRow matmuls at 0.5 cycles/row), which
cuts projection PE time 1.33x below bf16 while LOWERING error vs bf16
(residual ~2^-8 vs 2^-9, no first-order term dropped). The 32x weight
pre-scale keeps the lo-residuals out of fp8's subnormal floor; the scale
folds into the exp scale (/1024) and the l-ones constant (32.0).
Attention stays f32r exactly as v3. xT half-block tiles rotate through a
3-buffer pool (fp8 variants are 1.5x bf16 bytes).

Sharding (unchanged from v2/v3): core c handles batch b=c//2 and 1024 of
2048 query rows, interleaved per GROUPS for causal load balance; K/V for
the whole batch item are projected on both cores of a pair.
"""

import sys

import numpy as np

for _p in ("/opt/trn_rl_repo", "/root/.axon_site/_ro/trn_rl_repo"):
    if _p not in sys.path:
        sys.path.append(_p)

B, S, D = 4, 2048, 1024
P = 128
QL = 1024  # query rows per core
NSLOT = 8  # query slots (128 rows each) per core
NHB = 8  # key half-blocks of 256
NS = 24  # fp8 variant slices per contraction (3 per 128-deep k-tile)
GROUPS = {
    0: (15, 12, 11, 8, 7, 4, 3, 0),  # slot -> global q-tile, even cores
    1: (14, 13, 10, 9, 6, 5, 2, 1),  # odd cores
}
SCALE = 1.0 / np.sqrt(np.float32(D))

_cached = {}


def _build_bass():
    import concourse.bacc as bacc
    import concourse.mybir as mybir
    import concourse.tile as tile
    from contextlib import ExitStack

    f32 = mybir.dt.float32
    f32r = mybir.dt.float32r
    fp8 = mybir.dt.float8e4
    DR = mybir.MatmulPerfMode.DoubleRow

    nc = bacc.Bacc("TRN2")
    # Host-side layouts (partition-major, every DMA a contiguous [128,N]):
    #   xT:    [128, hb*(24*256) + s*256 + c]   (x^T fp8 slices s=3*dt+v)
    #   xqT:   [128, sb*(24*512) + s*512 + c]   (own q rows^T, 2 sb blocks)
    #   W*:    [128, s*1024 + e]                (weight fp8 variant slices)
    #   masks: [128, (2*hb+kt)*128 + c]         (finishing slot's causal tiles)
    xt_d = nc.declare_dram_parameter("xT", [P, NHB * NS * 256], fp8, isOutput=False)
    xq_d = nc.declare_dram_parameter("xqT", [P, 2 * NS * 512], fp8, isOutput=False)
    wq_d = nc.declare_dram_parameter("Wq", [P, NS * D], fp8, isOutput=False)
    wk_d = nc.declare_dram_parameter("Wk", [P, NS * D], fp8, isOutput=False)
    wv_d = nc.declare_dram_parameter("Wv", [P, NS * D], fp8, isOutput=False)
    masks_d = nc.declare_dram_parameter("masks", [P, 16 * P], f32r, isOutput=False)
    out_d = nc.declare_dram_parameter("out", [QL, D], f32, isOutput=True)

    ET = D // P  # 8 e-tiles

    with tile.TileContext(nc, pool_alloc_mode="queue") as tc, ExitStack() as top:
        cpool = top.enter_context(tc.tile_pool(name="const", bufs=1))
        ones_f = cpool.tile([P, 2], f32)
        nc.gpsimd.memset(ones_f, 32.0)  # folds the 32x of v~ into l
        ones = cpool.tile([P, 2], f32r)
        nc.vector.tensor_copy(ones, ones_f)

        # Persistent SBUF residents.
        qT_pool = top.enter_context(tc.tile_pool(name="qT", bufs=1))
        qT = [qT_pool.tile([P, QL], f32r, name=f"qT{e}") for e in range(ET)]
        wkv_pool = top.enter_context(tc.tile_pool(name="wkv", bufs=1))
        wk_sb = wkv_pool.tile([P, NS, D], fp8, name="wk")
        wv_sb = wkv_pool.tile([P, NS, D], fp8, name="wv")
        mpool = top.enter_context(tc.tile_pool(name="masks", bufs=1))
        masks_sb = mpool.tile([P, 16 * P], f32r)
        acc_pool = top.enter_context(tc.tile_pool(name="acc", bufs=1))
        O_sb = [acc_pool.tile([P, D], f32, name=f"O{j}") for j in range(NSLOT)]
        l_sb = acc_pool.tile([P, NSLOT], f32)
        # Rotating x^T half-block tiles (fp8 variants, 6KB/partition each).
        xTp = top.enter_context(tc.tile_pool(name="xT", bufs=3))

        # ---------------- Phase Q: qT projection ----------------
        with ExitStack() as pq_scope:
            wq_pool = pq_scope.enter_context(tc.tile_pool(name="wq", bufs=1))
            wq_sb = wq_pool.tile([P, NS, D], fp8)
            xq_pool = pq_scope.enter_context(tc.tile_pool(name="xq", bufs=1))
            xq_sb = [xq_pool.tile([P, NS, 512], fp8, name=f"xq{sb}") for sb in range(2)]

            # DMA issue order = data-need order; chunks so compute starts
            # as soon as the first chunk lands.
            nc.sync.dma_start(xq_sb[0], xq_d[:, 0 : NS * 512])
            nc.sync.dma_start(wq_sb, wq_d[:, :])
            nc.sync.dma_start(xq_sb[1], xq_d[:, NS * 512 : 2 * NS * 512])
            nc.sync.dma_start(wk_sb, wk_d[:, :])
            nc.sync.dma_start(wv_sb, wv_d[:, :])
            nc.sync.dma_start(masks_sb, masks_d[:, :])

            ppq = pq_scope.enter_context(tc.tile_pool(name="ppq", bufs=6, space="PSUM"))
            for sb in range(2):
                for et in range(ET):
                    pq = ppq.tile([P, 512], f32, tag="ppq", name="pq")
                    for j in range(0, NS, 2):
                        nc.tensor.matmul(
                            pq,
                            lhsT=wq_sb[:, j : j + 2, et * P : (et + 1) * P],
                            rhs=xq_sb[sb][:, j : j + 2, :],
                            start=(j == 0),
                            stop=(j == NS - 2),
                            perf_mode=DR,
                        )
                    nc.scalar.copy(qT[et][:, sb * 512 : (sb + 1) * 512], pq)

        # ---------------- Main loop: fused K/V projection + attention ----------------
        with ExitStack() as mn:
            kv_pool = mn.enter_context(tc.tile_pool(name="kv", bufs=2))
            wt_pool = mn.enter_context(tc.tile_pool(name="wt", bufs=4))
            fin_pool = mn.enter_context(tc.tile_pool(name="fin", bufs=2))
            rec_pool = mn.enter_context(tc.tile_pool(name="rec", bufs=2))
            ps_pool = mn.enter_context(tc.tile_pool(name="ps", bufs=7, space="PSUM"))
            lp_pool = mn.enter_context(tc.tile_pool(name="lp", bufs=1, space="PSUM"))

            for hb in range(NHB):
                n = NSLOT - hb  # active slot prefix length
                xT_hb = xTp.tile([P, NS, 256], fp8, tag="xT", name=f"xT{hb}")
                nc.sync.dma_start(xT_hb, xt_d[:, hb * NS * 256 : (hb + 1) * NS * 256])

                # kT[e, s] for this half-block: 8 tiles [128, 256]
                kTs = []
                for et in range(ET):
                    pk = ps_pool.tile([P, 512], f32, tag="ps", name="pk")
                    for j in range(0, NS, 2):
                        nc.tensor.matmul(
                            pk[:, 0:256],
                            lhsT=wk_sb[:, j : j + 2, et * P : (et + 1) * P],
                            rhs=xT_hb[:, j : j + 2, :],
                            start=(j == 0),
                            stop=(j == NS - 2),
                            perf_mode=DR,
                        )
                    kt_sb = kv_pool.tile([P, 256], f32r, tag=f"kT{et}", name=f"kT{et}")
                    nc.scalar.copy(kt_sb, pk[:, 0:256])
                    kTs.append(kt_sb)

                # v[s, e] for this half-block: 2 tiles [128, 1024]
                vs = []
                for st in range(2):
                    v_sb = kv_pool.tile([P, D], f32r, tag=f"v{st}", name=f"v{st}")
                    for eh in range(2):
                        pv = ps_pool.tile([P, 512], f32, tag="ps", name="pv")
                        for j in range(0, NS, 2):
                            nc.tensor.matmul(
                                pv,
                                lhsT=xT_hb[:, j : j + 2, st * P : (st + 1) * P],
                                rhs=wv_sb[:, j : j + 2, eh * 512 : (eh + 1) * 512],
                                start=(j == 0),
                                stop=(j == NS - 2),
                                perf_mode=DR,
                            )
                        nc.scalar.copy(v_sb[:, eh * 512 : (eh + 1) * 512], pv)
                    vs.append(v_sb)

                # scoresT + exp for the two key tiles of this half-block.
                width = P * n
                wpad = max(width, 256)  # keep moving dim >= 256 for f32r speed
                wts = []
                for kt in range(2):
                    wt = wt_pool.tile([P, D], f32r, tag="wt", name="wt")
                    for c0 in range(0, wpad, 512):
                        cw = min(512, wpad - c0)
                        sp = ps_pool.tile([P, 512], f32, tag="ps", name="sp")
                        for et in range(ET):
                            nc.tensor.matmul(
                                sp[:, 0:cw],
                                lhsT=kTs[et][:, kt * P : (kt + 1) * P],
                                rhs=qT[et][:, c0 : c0 + cw],
                                start=(et == 0),
                                stop=(et == ET - 1),
                            )
                        # q~ k~ = 1024 qk, so fold /1024 into the exp scale.
                        nc.scalar.activation(
                            wt[:, c0 : c0 + cw],
                            sp[:, 0:cw],
                            mybir.ActivationFunctionType.Exp,
                            scale=float(SCALE / 1024.0),
                        )
                    wts.append(wt)
                # Causal mask: only the finishing slot (j = n-1) is partial.
                for kt in range(2):
                    nc.vector.tensor_mul(
                        wts[kt][:, (n - 1) * P : n * P],
                        wts[kt][:, (n - 1) * P : n * P],
                        masks_sb[:, (2 * hb + kt) * P : (2 * hb + kt + 1) * P],
                    )

                # O_j += w^T V (PSUM-accumulated over the 2 key tiles), l_j += w^T 1.
                for j in range(n):
                    last = j == n - 1
                    if last:
                        lp = lp_pool.tile([P, 2], f32, tag="lp", name="lp")
                        for kt in range(2):
                            nc.tensor.matmul(
                                lp,
                                lhsT=wts[kt][:, j * P : (j + 1) * P],
                                rhs=ones,
                                start=(kt == 0),
                                stop=(kt == 1),
                            )
                        lcol = l_sb[:, j : j + 1]
                        if hb == 0:
                            nc.vector.tensor_copy(lcol, lp[:, 0:1])
                        else:
                            nc.vector.tensor_add(lcol, lcol, lp[:, 0:1])
                        rec = rec_pool.tile([P, 1], f32, tag="rc", name="rec")
                        nc.vector.reciprocal(rec, l_sb[:, j : j + 1])
                        fo = fin_pool.tile([P, D], f32, tag="fo", name="fo")
                    for eh in range(2):
                        op = ps_pool.tile([P, 512], f32, tag="ps", name="op")
                        for kt in range(2):
                            nc.tensor.matmul(
                                op,
                                lhsT=wts[kt][:, j * P : (j + 1) * P],
                                rhs=vs[kt][:, eh * 512 : (eh + 1) * 512],
                                start=(kt == 0),
                                stop=(kt == 1),
                            )
                        dst = O_sb[j][:, eh * 512 : (eh + 1) * 512]
                        if hb == 0:
                            nc.vector.tensor_copy(dst, op)
                        else:
                            nc.vector.tensor_add(dst, dst, op)
                        if last:
                            # Slot finished: normalize + store this half now.
                            cwf = 512
                            for f0 in range(eh * 512, (eh + 1) * 512, cwf):
                                nc.scalar.activation(
                                    fo[:, f0 : f0 + cwf],
                                    O_sb[j][:, f0 : f0 + cwf],
                                    mybir.ActivationFunctionType.Copy,
                                    scale=rec[:, 0:1],
                                )
                                nc.sync.dma_start(
                                    out_d[j * P : (j + 1) * P, f0 : f0 + cwf],
                                    fo[:, f0 : f0 + cwf],
                                )
                    if not last:
                        lp = lp_pool.tile([P, 2], f32, tag="lp", name="lp")
                        for kt in range(2):
                            nc.tensor.matmul(
                                lp,
                                lhsT=wts[kt][:, j * P : (j + 1) * P],
                                rhs=ones,
                                start=(kt == 0),
                                stop=(kt == 1),
                            )
                        lcol = l_sb[:, j : j + 1]
                        if hb == 0:
                            nc.vector.tensor_copy(lcol, lp[:, 0:1])
                        else:
                            nc.vector.tensor_add(lcol, lcol, lp[:, 0:1])

    nc.compile()
    return nc


def _fp8_variants(a):
    """a: [K, N] f32 -> (hi, lo16, hi16) f32 arrays (fp8-rounded values)."""
    import ml_dtypes

    f8 = ml_dtypes.float8_e4m3fn

    def q8(v):
        return v.astype(f8).astype(np.float32)

    hi = q8(a)
    lo16 = q8(16.0 * (a - hi))
    hi16 = q8(hi / 16.0)
    return hi, lo16, hi16


def _interleave(hi, lo16, hi16, ncols):
    """[K, ncols] variants -> [P, NS, ncols] fp8 in matmul slice order."""
    import ml_dtypes

    f8 = ml_dtypes.float8_e4m3fn
    KT = hi.shape[0] // P
    out = np.empty((P, 3 * KT, ncols), np.float32)
    for t in range(KT):
        sl = slice(t * P, (t + 1) * P)
        out[:, 3 * t + 0] = hi[sl]
        out[:, 3 * t + 1] = lo16[sl]
        out[:, 3 * t + 2] = hi16[sl]
    return np.ascontiguousarray(out.astype(f8))


def _host_inputs(x, Wq, Wk, Wv):
    # Weights: pre-scale by 32 so fp8 lo-residuals clear the subnormal
    # floor; the 32x on q/k/v folds into exp scale and the l-ones const.
    # Slot pairing: x slots are (hi, lo16, hi16); W slots must be
    # (hi, hi16, lo16) so slot-wise products give the 3 cross terms
    # x_hi*W_hi + x_lo16*W_hi16 + x_hi16*W_lo16.
    w_h = {}
    for name, w in (("Wq", Wq), ("Wk", Wk), ("Wv", Wv)):
        hi, lo16, hi16 = _fp8_variants(32.0 * w)
        w_h[name] = _interleave(hi, hi16, lo16, D).reshape(P, NS * D)

    in_maps = []
    for c in range(8):
        b, par = c // 2, c % 2
        groups = GROUPS[par]
        xb = x[b]  # [S, D]
        xT = np.ascontiguousarray(xb.T)  # [D, S]
        hi, lo16, hi16 = _fp8_variants(xT)
        # xT: per hb, [P, NS, 256] slices side by side
        xt_full = _interleave(hi, lo16, hi16, S).reshape(P, NS, NHB, 256)
        xt_h = np.ascontiguousarray(
            xt_full.transpose(0, 2, 1, 3).reshape(P, NHB * NS * 256)
        )
        # own query rows (slot-ordered), transposed: 2 sb blocks of [P, NS, 512]
        rows = np.concatenate([np.arange(P * g, P * g + P) for g in groups])
        xqT = np.ascontiguousarray(xb[rows].T)  # [D, QL]
        qhi, qlo16, qhi16 = _fp8_variants(xqT)
        xq_full = _interleave(qhi, qlo16, qhi16, QL).reshape(P, NS, 2, 512)
        xq_h = np.ascontiguousarray(
            xq_full.transpose(0, 2, 1, 3).reshape(P, 2 * NS * 512)
        )
        # masks: [128, (2*hb+kt)*128 + c]; finishing slot j=7-hb, K=2hb+kt
        masks = np.zeros((P, 16 * P), np.float32)
        for hb in range(NHB):
            j = 7 - hb
            g = groups[j]
            for kt in range(2):
                K = 2 * hb + kt
                kg = P * K + np.arange(P)[:, None]
                qg = P * g + np.arange(P)[None, :]
                masks[:, (2 * hb + kt) * P : (2 * hb + kt + 1) * P] = (
                    kg <= qg
                ).astype(np.float32)
        in_maps.append(
            {
                "xT": xt_h,
                "xqT": xq_h,
                "Wq": w_h["Wq"],
                "Wk": w_h["Wk"],
                "Wv": w_h["Wv"],
                "masks": masks,
            }
        )
    return in_maps


def kernel(x, Wq, Wk, Wv):
    from concourse.bass_utils import run_bass_kernel_spmd

    x = np.asarray(x, dtype=np.float32)
    Wq = np.ascontiguousarray(np.asarray(Wq, dtype=np.float32))
    Wk = np.ascontiguousarray(np.asarray(Wk, dtype=np.float32))
    Wv = np.ascontiguousarray(np.asarray(Wv, dtype=np.float32))

    if "nc" not in _cached:
        _cached["nc"] = _build_bass()
    nc = _cached["nc"]

    in_maps = _host_inputs(x, Wq, Wk, Wv)
    res = run_bass_kernel_spmd(nc, in_maps, core_ids=list(range(8)))
    _cached["last_result"] = res

    out = np.zeros((B, S, D), np.float32)
    for c in range(8):
        b, par = c // 2, c % 2
        oc = res.results[c]["out"]
        for j, g in enumerate(GROUPS[par]):
            out[b, P * g : P * g + P] = oc[P * j : P * (j + 1)]
    return out
